# revision 24
# baseline (speedup 1.0000x reference)
"""AttnDecoderRNN fused kernel for 8 Trainium2 NeuronCores.

Strategy (SPMD, one program on all 8 cores; per-core behavior differs only
through the data each core receives):
  - GRU step: gate-output (h) sharded across cores; each core computes
    decT_k [128, 32] for its h-shard, AllGather -> full decT [1024, 32].
  - Attention: u = dec @ W_attn[:, H:] computed full-batch in f32r, then a
    one-hot select matmul picks this core's 4 batches; scores/softmax/context
    computed per-core for its own 4 batches (enc slices are host-sharded).
    The constant-per-row term dec.(W1 dec + b_attn) is dropped: softmax is
    shift-invariant so it cancels exactly.
  - comb (attn_combine) per-core for 4 batches (bf16 weights), AllGather ->
    full comb [32, 1024].
  - Output projection: vocab-sharded (4000 cols/core) in bf16 with b_out
    folded in via an augmented contraction chunk; log_softmax normalizer via
    AllReduce of per-core sum(exp(logits)) (logits are O(+-4): no max needed).
Host side: shard/transpose/cast inputs, assemble outputs from the 8 cores.
"""
import sys
import numpy as np

sys.path.insert(0, "/opt/trn_rl_repo")

import concourse.bacc as bacc
import concourse.mybir as mybir
import concourse.tile as tile
from concourse.bass_utils import run_bass_kernel_spmd

B, S, I, H, E, V = 32, 128, 1024, 1024, 1024, 32000
NC = 8
BL = B // NC          # 4 local batches
HL = H // NC          # 128 h-shard
VL = V // NC          # 4000 vocab shard
VB = 8                # vocab blocks per core
VBW = VL // VB        # 500 cols per block
KC = I // 128         # 8 contraction chunks of 128
KCA = KC + 1          # + bias-augmented chunk
F32 = mybir.dt.float32
F32R = mybir.dt.float32r
BF16 = mybir.dt.bfloat16

# precision knobs (flip to F32 if accuracy demands; costs PE time)
GRU_DT = F32R
U_DT = F32R


def _re(ap, pat, **kw):
    return ap.rearrange(pat, **kw)


def build_program(sim_collectives=False):
    nc = bacc.Bacc("TRN2", target_bir_lowering=False, debug=False,
                   num_devices=NC, enable_asserts=False)
    d = {}
    def din(name, shape, dt):
        d[name] = nc.dram_tensor(name, shape, dt, kind="ExternalInput").ap()
        return d[name]

    x0t = din("x0t", [KCA * 128, B], F32)
    h0t = din("h0t", [KCA * 128, B], F32)
    wih = din("wih", [KCA * 128, 3 * HL], F32)
    whh = din("whh", [KCA * 128, 3 * HL], F32)
    h0s = din("h0s", [B, HL], F32)
    ident = din("ident", [128, 128], F32)
    sel = din("sel", [B, BL], F32)
    w2 = din("w2", [H, E], F32)
    enc_t = din("enc_t", [BL, E, S], F32)
    enc_s = din("enc_s", [S, BL, E], F32)
    x0t_loc = din("x0t_loc", [KCA * 128, BL], np.float32 and BF16)
    wc_t = din("wc_t", [(2 * KC + 1) * 128, I], BF16)
    wout_t = din("wout_t", [VB, KCA * 128, VBW], BF16)
    aug_ones = din("aug_ones", [128, B], BF16)

    attn_out = nc.dram_tensor("attn_out", [BL, S], F32, kind="ExternalOutput").ap()
    out = nc.dram_tensor("out", [B, VL], F32, kind="ExternalOutput").ap()

    AF = mybir.ActivationFunctionType
    with tile.TileContext(nc) as tc:
        with tc.tile_pool(name="wp", bufs=1) as wp, \
             tc.tile_pool(name="wop", bufs=2) as wop, \
             tc.tile_pool(name="sm", bufs=1) as sm, \
             tc.tile_pool(name="cxp", bufs=2) as cxp, \
             tc.tile_pool(name="outp", bufs=3) as outp, \
             tc.tile_pool(name="ejp", bufs=2) as ejp, \
             tc.tile_pool(name="ps", bufs=8, space="PSUM") as ps, \
             tc.tile_pool(name="dram", bufs=1, space="DRAM") as dram:

            # ---- persistent loads ----
            x_sb = wp.tile([128, KCA, B], GRU_DT)
            h_sb = wp.tile([128, KCA, B], GRU_DT)
            wih_sb = wp.tile([128, KCA, 3 * HL], GRU_DT)
            whh_sb = wp.tile([128, KCA, 3 * HL], GRU_DT)
            nc.sync.dma_start(x_sb[:], _re(x0t, "(c p) m -> p c m", p=128).bitcast(GRU_DT))
            nc.sync.dma_start(h_sb[:], _re(h0t, "(c p) m -> p c m", p=128).bitcast(GRU_DT))
            nc.sync.dma_start(wih_sb[:], _re(wih, "(c p) n -> p c n", p=128).bitcast(GRU_DT))
            nc.sync.dma_start(whh_sb[:], _re(whh, "(c p) n -> p c n", p=128).bitcast(GRU_DT))
            h0s_sb = sm.tile([B, HL], F32)
            nc.sync.dma_start(h0s_sb[:], h0s)
            id_sb = sm.tile([128, 128], F32)
            nc.sync.dma_start(id_sb[:], ident)
            sel_sb = sm.tile([B, BL], U_DT)
            nc.sync.dma_start(sel_sb[:], sel.bitcast(U_DT))
            enct_sb = wp.tile([128, BL, KC, S], F32)
            nc.sync.dma_start(enct_sb[:], _re(enc_t, "b (c p) s -> p b c s", p=128))
            encs_sb = wp.tile([128, BL, E], F32R)
            nc.sync.dma_start(encs_sb[:], _re(enc_s, "s b e -> s b e").bitcast(F32R))
            xloc_sb = sm.tile([128, KCA, BL], BF16)
            nc.sync.dma_start(xloc_sb[:], _re(x0t_loc, "(c p) m -> p c m", p=128))
            ao_sb = sm.tile([128, B], BF16)
            nc.sync.dma_start(ao_sb[:], aug_ones)
            # big weight streams, issued early on the sync HWDGE ring so they
            # are never queued behind collective-dependent transfers
            w2_sb = wp.tile([128, KC, E], U_DT)
            for c in range(KC):
                nc.sync.dma_start(w2_sb[:, c, :],
                                  _re(w2, "(c p) e -> p c e", p=128)[:, c, :].bitcast(U_DT))
            wc_sb = wp.tile([128, 2 * KC + 1, I], BF16)
            for c in range(2 * KC + 1):
                nc.sync.dma_start(wc_sb[:, c, :],
                                  _re(wc_t, "(c p) n -> p c n", p=128)[:, c, :])
            wo_tiles = []
            for vb in range(VB):
                wo = wop.tile([128, KCA, VBW], BF16, tag="wo", name=f"wo{vb}")
                nc.sync.dma_start(wo[:], _re(wout_t[vb], "(c p) n -> p c n", p=128))
                wo_tiles.append(wo)

            # ---- GRU: xg/hg [B, 3*HL] ----
            xg = ps.tile([B, 3 * HL], F32, tag="ps")
            hg = ps.tile([B, 3 * HL], F32, tag="ps")
            for c in range(KCA):
                nc.tensor.matmul(xg[:], x_sb[:, c, :], wih_sb[:, c, :],
                                 start=(c == 0), stop=(c == KCA - 1))
            for c in range(KCA):
                nc.tensor.matmul(hg[:], h_sb[:, c, :], whh_sb[:, c, :],
                                 start=(c == 0), stop=(c == KCA - 1))
            hg_sb = sm.tile([B, 3 * HL], F32)
            nc.vector.tensor_copy(hg_sb[:], hg[:])
            rz = sm.tile([B, 2 * HL], F32)
            nc.vector.tensor_add(rz[:], xg[:, 0:2 * HL], hg_sb[:, 0:2 * HL])
            r_sb = sm.tile([B, HL], F32)
            z_sb = sm.tile([B, HL], F32)
            nc.scalar.activation(r_sb[:], rz[:, 0:HL], AF.Sigmoid)
            nc.scalar.activation(z_sb[:], rz[:, HL:2 * HL], AF.Sigmoid)
            rn = sm.tile([B, HL], F32)
            nc.vector.tensor_mul(rn[:], hg_sb[:, 2 * HL:3 * HL], r_sb[:])
            npre = sm.tile([B, HL], F32)
            nc.vector.tensor_add(npre[:], xg[:, 2 * HL:3 * HL], rn[:])
            n_sb = sm.tile([B, HL], F32)
            nc.scalar.activation(n_sb[:], npre[:], AF.Tanh)
            t1 = sm.tile([B, HL], F32)
            nc.vector.tensor_sub(t1[:], h0s_sb[:], n_sb[:])
            t2 = sm.tile([B, HL], F32)
            nc.vector.tensor_mul(t2[:], z_sb[:], t1[:])
            dec_sb = sm.tile([B, HL], F32)
            nc.vector.tensor_add(dec_sb[:], n_sb[:], t2[:])

            # decT [128, 32]
            dT_ps = ps.tile([HL, B], F32, tag="ps")
            nc.tensor.transpose(dT_ps[:], dec_sb[:], id_sb[0:B, 0:B])
            dT_sb = sm.tile([HL, B], F32)
            nc.vector.tensor_copy(dT_sb[:], dT_ps[:])

            # AllGather dec
            agd_i = dram.tile([HL, B], F32)
            agd_o = dram.tile([H, B], F32, addr_space="Shared")
            nc.gpsimd.dma_start(agd_i[:], dT_sb[:])
            if sim_collectives:
                nc.gpsimd.dma_start(agd_o[0:HL, :], agd_i[:])
            else:
                nc.gpsimd.collective_compute(
                    "AllGather", mybir.AluOpType.bypass,
                    replica_groups=[list(range(NC))],
                    ins=[agd_i.opt()], outs=[agd_o.opt()])
            dF_sb = wp.tile([128, KC, B], U_DT)
            nc.scalar.dma_start(dF_sb[:], _re(agd_o[:], "(c p) m -> p c m", p=128).bitcast(U_DT))

            # u full [B, E] in f32r
            uf_sb = sm.tile([B, E], U_DT)
            for half in range(2):
                u_ps = ps.tile([B, 512], F32, tag="ps", name=f"u_ps{half}")
                for c in range(KC):
                    nc.tensor.matmul(u_ps[:], dF_sb[:, c, :],
                                     w2_sb[:, c, 512 * half:512 * (half + 1)],
                                     start=(c == 0), stop=(c == KC - 1))
                nc.vector.tensor_copy(uf_sb[:, 512 * half:512 * (half + 1)], u_ps[:])
            # select local 4 batches: u_loc [BL, E]
            ul_sb = sm.tile([BL, E], F32)
            for half in range(2):
                ul_ps = ps.tile([BL, 512], F32, tag="ps", name=f"ul_ps{half}")
                nc.tensor.matmul(ul_ps[:], sel_sb[:], uf_sb[:, 512 * half:512 * (half + 1)],
                                 start=True, stop=True)
                nc.vector.tensor_copy(ul_sb[:, 512 * half:512 * (half + 1)], ul_ps[:])
            # uT [128, KC, BL]
            uT_ps = ps.tile([128, KC * BL], F32, tag="ps")
            for c in range(KC):
                nc.tensor.transpose(uT_ps[:, BL * c:BL * (c + 1)],
                                    ul_sb[:, 128 * c:128 * (c + 1)], id_sb[0:BL, 0:BL])
            uT_sb = sm.tile([128, KC, BL], F32)
            nc.vector.tensor_copy(uT_sb[:], _re(uT_ps[:], "p (c j) -> p c j", c=KC))

            # scores: per local batch, fp32, contraction over full E
            scs_sb = sm.tile([1, BL, S], F32)
            for j in range(BL):
                sc_ps = ps.tile([1, S], F32, tag="ps", name=f"sc_ps{j}")
                for c in range(KC):
                    nc.tensor.matmul(sc_ps[:], uT_sb[:, c, j:j + 1], enct_sb[:, j, c, :],
                                     start=(c == 0), stop=(c == KC - 1))
                nc.vector.tensor_copy(scs_sb[0:1, j, :], sc_ps[:])
            # softmax over S on one partition
            mx = sm.tile([1, BL], F32)
            nc.vector.tensor_reduce(mx[:], scs_sb[0:1, :, :], mybir.AxisListType.X,
                                    mybir.AluOpType.max)
            nmx = sm.tile([1, BL], F32)
            nc.vector.tensor_scalar_mul(nmx[:], mx[:], -1.0)
            w_row = sm.tile([1, BL, S], F32)
            sums = sm.tile([1, BL], F32)
            for j in range(BL):
                nc.scalar.activation(w_row[0:1, j, :], scs_sb[0:1, j, :], AF.Exp,
                                     bias=nmx[0:1, j:j + 1], accum_out=sums[0:1, j:j + 1])
            rs = sm.tile([1, BL], F32)
            nc.vector.reciprocal(rs[:], sums[:])
            for j in range(BL):
                nc.vector.tensor_scalar_mul(w_row[0:1, j, :], w_row[0:1, j, :],
                                            rs[0:1, j:j + 1])
            nc.scalar.dma_start(attn_out, w_row[0:1, :, :])

            # wT [S, BL] for context
            wT_ps = ps.tile([S, BL], F32, tag="ps")
            for j in range(BL):
                nc.tensor.transpose(wT_ps[:, j:j + 1], w_row[0:1, j, :], id_sb[0:1, 0:1])
            wT_sb = sm.tile([S, BL], U_DT)
            nc.vector.tensor_copy(wT_sb[:], wT_ps[:])

            # context per batch (f32r), rows [1, E] then transpose-dance
            ctxT_ps = ps.tile([128, KC * BL], F32, tag="ps")
            for j in range(BL):
                for half in range(2):
                    cx_ps = ps.tile([1, 512], F32, tag="ps", name=f"cx_ps{j}_{half}")
                    nc.tensor.matmul(cx_ps[:], wT_sb[:, j:j + 1],
                                     encs_sb[:, j, 512 * half:512 * (half + 1)],
                                     start=True, stop=True)
                    cxr = cxp.tile([1, 512], F32, tag="cxr", name=f"cxr{j}_{half}")
                    if half == 0:
                        nc.vector.tensor_copy(cxr[:], cx_ps[:])
                    else:
                        nc.scalar.activation(cxr[:], cx_ps[:], AF.Copy)
                    for ci in range(4):
                        c = 4 * half + ci
                        nc.tensor.transpose(ctxT_ps[:, BL * c + j:BL * c + j + 1],
                                            cxr[0:1, 128 * ci:128 * (ci + 1)],
                                            id_sb[0:1, 0:1])
            ctxT_sb = sm.tile([128, KC, BL], BF16)
            nc.vector.tensor_copy(ctxT_sb[:], _re(ctxT_ps[:], "p (c j) -> p c j", c=KC))

            # comb [BL, I] bf16 weights, contraction (x-aug 9 + ctx 8 chunks).
            # x-part first: it has no dependency on attention, so the PE can
            # run those 18 matmuls early; ctx chunks continue the accumulation.
            cb_sb = sm.tile([BL, I], F32)
            NCMB = 2 * KC + 1
            cb_ps_t = [ps.tile([BL, 512], F32, tag="ps", name=f"cb_ps{h}")
                       for h in range(2)]
            for c in range(NCMB):
                lhsT = xloc_sb[:, c, :] if c < KCA else ctxT_sb[:, c - KCA, :]
                for half in range(2):
                    nc.tensor.matmul(cb_ps_t[half][:], lhsT,
                                     wc_sb[:, c, 512 * half:512 * (half + 1)],
                                     start=(c == 0), stop=(c == NCMB - 1))
            for half in range(2):
                nc.vector.tensor_copy(cb_sb[:, 512 * half:512 * (half + 1)],
                                      cb_ps_t[half][:])

            # AllGather comb -> [B, I]
            agc_i = dram.tile([BL, I], F32)
            agc_o = dram.tile([B, I], F32, addr_space="Shared")
            nc.gpsimd.dma_start(agc_i[:], cb_sb[:])
            if sim_collectives:
                nc.gpsimd.dma_start(agc_o[0:BL, :], agc_i[:])
            else:
                nc.gpsimd.collective_compute(
                    "AllGather", mybir.AluOpType.bypass,
                    replica_groups=[list(range(NC))],
                    ins=[agc_i.opt()], outs=[agc_o.opt()])
            cg_sb = sm.tile([B, KC, 128], F32)
            nc.scalar.dma_start(cg_sb[:], _re(agc_o[:], "b (c f) -> b c f", c=KC))
            # combT chunks [128, KC, B] bf16: 8 transposes into one PSUM tile,
            # then a single cast-copy out.
            cT_sb = sm.tile([128, KC, B], BF16)
            cT_ps = ps.tile([128, KC * B], F32, tag="ps")
            for c in range(KC):
                nc.tensor.transpose(cT_ps[:, B * c:B * (c + 1)], cg_sb[:, c, :],
                                    id_sb[0:B, 0:B])
            nc.vector.tensor_copy(cT_sb[:], _re(cT_ps[:], "p (c m) -> p c m", c=KC))

            # vocab projection, bf16, vocab-sharded; keep logits in PSUM
            se_sb = sm.tile([B, VB], F32)
            lg_tiles = []
            for vb in range(VB):
                wo = wo_tiles[vb]
                lg = ps.tile([B, VBW], F32, tag="ps", name=f"lg{vb}")
                for c in range(KCA):
                    lhsT = cT_sb[:, c, :] if c < KC else ao_sb[:]
                    nc.tensor.matmul(lg[:], lhsT, wo[:, c, :],
                                     start=(c == 0), stop=(c == KCA - 1))
                ej = ejp.tile([B, VBW], F32, tag="ej", name=f"ej{vb}")
                nc.scalar.activation(ej[:], lg[:], AF.Exp,
                                     accum_out=se_sb[:, vb:vb + 1])
                lg_tiles.append(lg)

            sloc = sm.tile([B, 1], F32)
            nc.vector.tensor_reduce(sloc[:], se_sb[:], mybir.AxisListType.X,
                                    mybir.AluOpType.add)
            ars_i = dram.tile([B, 1], F32)
            ars_o = dram.tile([B, 1], F32, addr_space="Shared")
            nc.gpsimd.dma_start(ars_i[:], sloc[:])
            if sim_collectives:
                nc.gpsimd.dma_start(ars_o[:], ars_i[:])
            else:
                nc.gpsimd.collective_compute(
                    "AllReduce", mybir.AluOpType.add,
                    replica_groups=[list(range(NC))],
                    ins=[ars_i.opt()], outs=[ars_o.opt()])
            s_sb = sm.tile([B, 1], F32)
            nc.scalar.dma_start(s_sb[:], ars_o[:])
            lnS = sm.tile([B, 1], F32)
            nc.scalar.activation(lnS[:], s_sb[:], AF.Ln)
            nls = sm.tile([B, 1], F32)
            nc.vector.tensor_scalar_mul(nls[:], lnS[:], -1.0)
            for vb in range(VB):
                ob = outp.tile([B, VBW], F32, tag="ob", name=f"ob{vb}")
                if vb % 2 == 0:
                    nc.vector.tensor_scalar_add(ob[:], lg_tiles[vb][:], nls[:])
                else:
                    nc.scalar.activation(ob[:], lg_tiles[vb][:], AF.Identity, bias=nls[:])
                nc.scalar.dma_start(out[:, VBW * vb:VBW * (vb + 1)], ob[:])
    nc.compile()
    return nc


_prog = None


def _get_prog():
    global _prog
    if _prog is None:
        _prog = build_program()
    return _prog


def _prep_inputs(x, encoder_outputs, hidden, W_ih, W_hh, b_ih, b_hh,
                 W_attn, b_attn, W_comb, b_comb, W_out, b_out):
    f32 = np.float32
    x0 = np.asarray(x, f32)[0]              # [B, I]
    h0 = np.asarray(hidden, f32)[0]         # [B, H]
    enc = np.asarray(encoder_outputs, f32)  # [S, B, E]
    W_ih = np.asarray(W_ih, f32); W_hh = np.asarray(W_hh, f32)
    b_ih = np.asarray(b_ih, f32); b_hh = np.asarray(b_hh, f32)
    W2 = np.asarray(W_attn, f32)[:, H:]     # [H, E]
    W_comb = np.asarray(W_comb, f32); b_comb = np.asarray(b_comb, f32)
    W_out = np.asarray(W_out, f32); b_out = np.asarray(b_out, f32)

    def aug_cols(mat_t, bias_row):
        # mat_t [1024, N] -> [1152, N] with row 1024 = bias, rest 0
        n = mat_t.shape[1]
        out = np.zeros((KCA * 128, n), f32)
        out[:I] = mat_t
        out[I] = bias_row
        return out

    x0t = aug_cols(x0.T, np.zeros(B, f32)); x0t[I] = 1.0
    h0t = aug_cols(h0.T, np.zeros(B, f32)); h0t[I] = 1.0
    ident = np.eye(128, dtype=f32)

    wc_t = np.zeros(((2 * KC + 1) * 128, I), f32)
    wc_t[:I] = W_comb[:, :I].T
    wc_t[I] = b_comb
    wc_t[KCA * 128:] = W_comb[:, I:].T
    wc_t = wc_t.astype(np.dtype("bfloat16") if hasattr(np, "bfloat16") else f32)

    import ml_dtypes
    bf16 = ml_dtypes.bfloat16
    wc_t = np.ascontiguousarray(wc_t).astype(bf16)

    in_maps = []
    for k in range(NC):
        hsl = slice(HL * k, HL * (k + 1))
        gate_rows = np.concatenate([np.arange(HL * k, HL * (k + 1)),
                                    np.arange(H + HL * k, H + HL * (k + 1)),
                                    np.arange(2 * H + HL * k, 2 * H + HL * (k + 1))])
        wih_k = aug_cols(W_ih[gate_rows].T, b_ih[gate_rows])
        whh_k = aug_cols(W_hh[gate_rows].T, b_hh[gate_rows])
        bsl = slice(BL * k, BL * (k + 1))
        sel = np.zeros((B, BL), f32)
        for j in range(BL):
            sel[BL * k + j, j] = 1.0
        enc_loc = enc[:, bsl, :]                                  # [S, BL, E]
        enc_t = np.ascontiguousarray(enc_loc.transpose(1, 2, 0))  # [BL, E, S]
        enc_s = np.ascontiguousarray(enc_loc)                     # [S, BL, E]
        x0t_loc = np.ascontiguousarray(x0t[:, bsl]).astype(bf16)  # [1152, BL]
        vsl = slice(VL * k, VL * (k + 1))
        wout_a = np.zeros((KCA * 128, VL), f32)
        wout_a[:I] = W_out[vsl].T
        wout_a[I] = b_out[vsl]
        wout_t = np.ascontiguousarray(
            wout_a.reshape(KCA * 128, VB, VBW).transpose(1, 0, 2)).astype(bf16)
        aug_ones = np.zeros((128, B), f32); aug_ones[0] = 1.0
        in_maps.append({
            "x0t": x0t, "h0t": h0t, "wih": wih_k, "whh": whh_k,
            "h0s": np.ascontiguousarray(h0[:, hsl]), "ident": ident,
            "sel": sel, "w2": np.ascontiguousarray(W2),
            "enc_t": enc_t, "enc_s": enc_s, "x0t_loc": x0t_loc,
            "wc_t": wc_t, "wout_t": wout_t,
            "aug_ones": aug_ones.astype(bf16),
        })
    return in_maps


def kernel(**inputs):
    in_maps = _prep_inputs(**inputs)
    prog = _get_prog()
    res = run_bass_kernel_spmd(prog, in_maps, core_ids=list(range(NC)))
    attn = np.concatenate([res.results[k]["attn_out"] for k in range(NC)], axis=0)
    out = np.concatenate([res.results[k]["out"] for k in range(NC)], axis=1)
    return out.astype(np.float32), attn.astype(np.float32)


if __name__ == "__main__":
    sys.path.insert(0, "/root/problem")
    import reference
    inputs = {k: np.asarray(v) for k, v in reference.setup_inputs().items()}
    o, a = kernel(**inputs)
    print("out", o.shape, "attn", a.shape)
